# revision 9
# baseline (speedup 1.0000x reference)
"""Trainium2 Bass kernel for nn_CustomLoss_11630771438153 (retrieval_knn).

Strategy v2: shard the database X row-wise across 8 NeuronCores, with the
database pre-sorted by ||x||^2 on the host. Each core computes the raw
inner-product score s' = tq2.T @ X_shard (tq2 = 2*(q@W).T, bf16) on the
TensorEngine -- a single matmul per 512-column PSUM bank, NO xnorm
accumulation pass (the sort makes per-unit xnorm nearly constant, so the
host folds it into the screening bounds analytically). The 3.2M score
elements per core are then screened by TWO engines in parallel, chunk
interleaved:

  - DVE `pool_max` (window 32): per-32-column block maxima of s',
    one instruction per 1024-col chunk covering both query blocks.
  - ScalarE `activation(Exp, bias=-U_q/T, scale=1/T, accum_out=...)`:
    per-512-column-group sumexp, giving log-sum-exp bounds
    max' <= U + T*ln(sum) <= max' + T*ln(512).

Both engines read PSUM directly (1 elem/cycle/lane each); running them
concurrently roughly halves the screening wall vs. the MAX8-only baseline,
and dropping the xnorm matmul pass halves TensorEngine time.

The host merges: every reported unit yields a sound upper bound UB on the
best true score in the unit and a sound lower bound LB witnessing one real
point; t_q = 16th largest LB guarantees >=16 points above it; all units
with UB >= t_q are rescanned exactly in fp32 (numpy), giving exact top-16
as long as the device-vs-exact deviation delta is covered (validated on the
rescanned units, with automatic escalation of delta when violated).
"""

import sys

sys.path.insert(0, "/opt/trn_rl_repo")

import ml_dtypes
import numpy as np

import concourse.tile as tile
from concourse import bacc, mybir
from concourse.bass_utils import run_bass_kernel_spmd

# Problem constants (hardcoded per the harness contract).
B = 256  # queries
D = 128  # feature dim
N = 100000  # database size
K = 16  # neighbors
TAU = 0.1
BETA = 1.0
LAMB = 1e-4
EPS = 1e-8

N_CORES = 8
N_CORE = N // N_CORES  # 12500 database rows per core
QB = B // 128  # 2 query blocks of 128

W_CHUNK = 1024  # columns per full chunk (per qblock)
N_FULL = 12  # full chunks per core (12*1024 = 12288)
REM = N_CORE - N_FULL * W_CHUNK  # 212 remainder columns
REM_PAD = 224  # padded remainder (multiple of 32)
N_PAD = N_FULL * W_CHUNK + REM_PAD  # 12512 padded per-core columns

POOLW = 32  # pool_max window (host rescan granularity for DVE chunks)
SGRP = 512  # scalar sumexp group width
T_SM = 0.75  # softmax screen temperature
U_PAD = 25.0  # safety pad on the per-query shift estimate
WARMUP_MM = 10  # dummy matmuls to trip the PE HAM clock gate to 2.4 GHz

# chunk ci -> screening engine: alternate DVE/ACT; remainder chunk on DVE.
# (chunk widths: N_FULL x 1024 + one REM_PAD chunk)
CHUNK_ENG = ["D" if ci % 2 == 0 else "A" for ci in range(N_FULL)] + ["D"]
N_CHUNKS = len(CHUNK_ENG)

# output column layout: per chunk, DVE -> 2w/32 block maxes, ACT -> 4 sums
_off = 0
CHUNK_OUT_OFF = []
for _ci in range(N_CHUNKS):
    _w = W_CHUNK if _ci < N_FULL else REM_PAD
    CHUNK_OUT_OFF.append(_off)
    _off += (2 * _w) // POOLW if CHUNK_ENG[_ci] == "D" else QB * (_w // SGRP)
OUTW = _off  # 6*64 + 6*4 + 14 = 422

_compiled = {}
LAST_EXEC_NS = None


def _build_kernel():
    """Build + compile the SPMD Bass program (identical on all cores)."""
    nc = bacc.Bacc(
        "TRN2", target_bir_lowering=False, debug=False, num_devices=N_CORES
    )
    f32 = mybir.dt.float32
    bf16 = mybir.dt.bfloat16

    xt = nc.dram_tensor("xt", [D, N_PAD], bf16, kind="ExternalInput").ap()
    tq2_in = nc.dram_tensor("tq2", [D, B], bf16, kind="ExternalInput").ap()
    # per-partition exp bias: col qb holds -U_q/T for query (qb*128 + p)
    negb_in = nc.dram_tensor("negb", [128, QB], f32, kind="ExternalInput").ap()
    cand = nc.dram_tensor("cand", [128, OUTW], f32, kind="ExternalOutput").ap()

    with tile.TileContext(nc) as tc:
        with (
            tc.tile_pool(name="const", bufs=1) as const_pool,
            tc.tile_pool(name="xchunk", bufs=4) as x_pool,
            tc.tile_pool(name="out", bufs=1) as out_pool,
            tc.tile_pool(name="psum", bufs=2, space="PSUM") as psum_pool,
        ):
            tq2 = const_pool.tile([D, B], bf16)
            nc.sync.dma_start(tq2[:], tq2_in[:])
            negb = const_pool.tile([128, QB], f32)
            nc.sync.dma_start(negb[:], negb_in[:])

            out_sb = out_pool.tile([128, OUTW], f32, name="out_sb")

            for ci in range(N_CHUNKS):
                w = W_CHUNK if ci < N_FULL else REM_PAD
                c0 = ci * W_CHUNK
                xc = x_pool.tile([D, w], bf16, tag="xc", name=f"xc{ci}")
                nc.sync.dma_start(xc[:], xt[:, c0 : c0 + w])

                # qb1 scores start at a 512-aligned (bank-aligned) offset so
                # every matmul output region is bank-exclusive
                qb_off = w if w % 512 == 0 else 512
                ps = psum_pool.tile(
                    [128, 2 * qb_off], f32, tag="ps", name=f"ps{ci}"
                )

                if ci == 0:
                    # PE warmup during the initial DMA window: back-to-back
                    # dummy matmuls open the HAM clock gate (1.2 -> 2.4 GHz).
                    for _ in range(WARMUP_MM):
                        nc.tensor.matmul(
                            ps[:, 0:128],
                            tq2[:, 0:128],
                            tq2[:, 0:128],
                            start=True,
                            stop=True,
                        )

                for qb in range(QB):
                    lhs = tq2[:, qb * 128 : (qb + 1) * 128]
                    for h0 in range(0, w, 512):
                        hw = min(512, w - h0)
                        nc.tensor.matmul(
                            ps[:, qb * qb_off + h0 : qb * qb_off + h0 + hw],
                            lhs,
                            xc[:, h0 : h0 + hw],
                            start=True,
                            stop=True,
                        )

                off = CHUNK_OUT_OFF[ci]
                if CHUNK_ENG[ci] == "D":
                    nblk_qb = w // POOLW
                    if w == qb_off:
                        view = ps[:, 0 : 2 * w].rearrange(
                            "p (a b) -> p a b", b=POOLW
                        )
                        nc.vector.tensor_reduce(
                            out_sb[:, off : off + 2 * nblk_qb],
                            view,
                            axis=mybir.AxisListType.X,
                            op=mybir.AluOpType.max,
                        )
                    else:
                        for qb in range(QB):
                            view = ps[
                                :, qb * qb_off : qb * qb_off + w
                            ].rearrange("p (a b) -> p a b", b=POOLW)
                            nc.vector.tensor_reduce(
                                out_sb[
                                    :,
                                    off + qb * nblk_qb : off + (qb + 1) * nblk_qb,
                                ],
                                view,
                                axis=mybir.AxisListType.X,
                                op=mybir.AluOpType.max,
                            )
                else:
                    ngrp = w // SGRP
                    for qb in range(QB):
                        for g in range(ngrp):
                            sl = ps[
                                :,
                                qb * qb_off + g * SGRP : qb * qb_off
                                + (g + 1) * SGRP,
                            ]
                            nc.scalar.activation(
                                sl,
                                sl,
                                mybir.ActivationFunctionType.Exp,
                                bias=negb[:, qb : qb + 1],
                                scale=1.0 / T_SM,
                                accum_out=out_sb[
                                    :,
                                    off + qb * ngrp + g : off + qb * ngrp + g + 1,
                                ],
                            )

            nc.sync.dma_start(cand[:], out_sb[:])

    nc.compile()
    return nc


def _get_compiled():
    if "nc" not in _compiled:
        _compiled["nc"] = _build_kernel()
    return _compiled["nc"]


def _softmax_f32(x):
    x = x.astype(np.float32)
    m = np.max(x, axis=1, keepdims=True)
    e = np.exp(x - m)
    return e / np.sum(e, axis=1, keepdims=True)


def _unit_tables():
    """Static per-core unit tables: (kind, qb, col0, width, valid_width).

    Column offsets are within the core's padded [0, N_PAD) range; validity
    clips to N_CORE. Returns a list of dicts per output column index.
    """
    units = []
    for ci in range(N_CHUNKS):
        w = W_CHUNK if ci < N_FULL else REM_PAD
        c0 = ci * W_CHUNK
        if CHUNK_ENG[ci] == "D":
            nblk = (2 * w) // POOLW
            for j in range(nblk):
                qb = (j * POOLW) // w
                local = (j * POOLW) % w
                col0 = c0 + local
                vw = max(0, min(POOLW, N_CORE - col0))
                units.append(("D", qb, col0, POOLW, vw))
        else:
            ngrp = w // SGRP
            for qb in range(QB):
                for g in range(ngrp):
                    col0 = c0 + g * SGRP
                    vw = max(0, min(SGRP, N_CORE - col0))
                    units.append(("A", qb, col0, SGRP, vw))
    return units


_UNITS = _unit_tables()


def kernel(q_batch, q_indices, X, W, pre_indices, pre_weights):
    q_batch = np.asarray(q_batch, dtype=np.float32)
    X = np.asarray(X, dtype=np.float32)
    W = np.asarray(W, dtype=np.float32)
    q_indices = np.asarray(q_indices)
    pre_indices = np.asarray(pre_indices)
    pre_weights = np.asarray(pre_weights, dtype=np.float32)

    # ---- host prep: xnorm-sort the database, shard, quantize ---------------
    xnorm = np.sum(X * X, axis=1, dtype=np.float32)  # [N]
    order = np.argsort(xnorm, kind="stable")  # ascending
    Xs = np.ascontiguousarray(X[order])  # [N, D] sorted
    xn_s = xnorm[order]  # [N]
    tq2m = 2.0 * (q_batch @ W)  # [B, D] fp32
    tq2_dev = np.ascontiguousarray(tq2m.T.astype(ml_dtypes.bfloat16))  # [D, B]

    # per-query shift U_q: max inner-product score over a subsample + pad
    sub = Xs[:: max(1, N // 4096)]
    U_q = (tq2m @ sub.T).max(axis=1).astype(np.float32) + np.float32(U_PAD)
    negb = np.ascontiguousarray(
        (-U_q / np.float32(T_SM)).reshape(QB, 128).T.astype(np.float32)
    )  # [128, QB]

    xt_full = np.zeros((D, N_CORES * N_PAD), dtype=ml_dtypes.bfloat16)
    xs_t = Xs.T.astype(ml_dtypes.bfloat16)  # [D, N]
    for c in range(N_CORES):
        xt_full[:, c * N_PAD : c * N_PAD + N_CORE] = xs_t[
            :, c * N_CORE : (c + 1) * N_CORE
        ]

    nc = _get_compiled()
    in_maps = []
    for c in range(N_CORES):
        in_maps.append(
            {
                "xt": np.ascontiguousarray(
                    xt_full[:, c * N_PAD : (c + 1) * N_PAD]
                ),
                "tq2": tq2_dev,
                "negb": negb,
            }
        )
    res = run_bass_kernel_spmd(nc, in_maps, core_ids=list(range(N_CORES)))
    global LAST_EXEC_NS
    if res.exec_time_ns is not None:
        LAST_EXEC_NS = res.exec_time_ns

    outs = [res.results[c]["cand"] for c in range(N_CORES)]  # [128, OUTW] each

    # ---- build per-unit bounds --------------------------------------------
    # unit arrays over all cores: shape [B, NU_total]
    nu = len(_UNITS)
    ub = np.full((B, N_CORES * nu), -np.inf, dtype=np.float64)
    lb = np.full((B, N_CORES * nu), -np.inf, dtype=np.float64)
    # metadata per global unit
    u_core = np.repeat(np.arange(N_CORES), nu)
    u_col0 = np.tile(np.array([u[2] for u in _UNITS]), N_CORES) + u_core * N_CORE
    u_vw = np.tile(np.array([u[4] for u in _UNITS]), N_CORES)
    u_kind = np.tile(np.array([u[0] for u in _UNITS]), N_CORES)
    u_qb = np.tile(np.array([u[1] for u in _UNITS]), N_CORES)

    # device values per unit, per query-in-block: dv[q_in_block, unit]
    lnT = np.float64(T_SM)
    ln_w = np.log(np.float64(SGRP))
    dev_vals = np.concatenate(
        [o.astype(np.float64) for o in outs], axis=1
    )  # [128, N_CORES*nu]

    # xnorm bounds per unit (sorted -> min/max at ends); pad cols excluded
    col0 = u_col0
    colend = u_col0 + u_vw - 1
    valid = u_vw > 0
    xmin2 = np.where(valid, xn_s[np.clip(col0, 0, N - 1)], 0.0)
    xmax2 = np.where(valid, xn_s[np.clip(colend, 0, N - 1)], 0.0)

    delta = 0.5  # device-vs-exact score deviation budget (escalated on fail)

    is_pool = u_kind == "D"
    is_act = ~is_pool
    partial = u_vw < np.where(is_pool, POOLW, SGRP)  # pad-touching units

    rows = np.arange(B)[:, None]

    for _attempt in range(5):
        for qb in range(QB):
            qsl = slice(qb * 128, (qb + 1) * 128)
            m = u_qb == qb
            mp = m & is_pool & valid
            ma = m & is_act & valid
            dv = dev_vals[:, mp]
            ub_q = np.full((128, nu * N_CORES), -np.inf)
            lb_q = np.full((128, nu * N_CORES), -np.inf)
            # pool units: dv = block max of s'
            ub_q[:, mp] = dv + delta - xmin2[mp][None, :]
            lb_q[:, mp] = dv - delta - xmax2[mp][None, :]
            # act units: dv = sumexp((s' - U_q)/T)
            dva = dev_vals[:, ma]
            with np.errstate(divide="ignore"):
                lns = np.log(dva)
            Uq = U_q[qsl].astype(np.float64)[:, None]
            ub_q[:, ma] = Uq + lnT * lns + delta - xmin2[ma][None, :]
            lb_q[:, ma] = Uq + lnT * (lns - ln_w) - delta - xmax2[ma][None, :]
            # pad-touching pool units: device max may come from a pad column
            # (score 0) -> LB is not witnessed by a real point
            lb_q[:, m & partial] = -np.inf
            ub[qsl] = ub_q
            lb[qsl] = lb_q

        # threshold: 16th largest LB per query -> >=16 real points above it
        t_q = -np.partition(-lb, K - 1, axis=1)[:, K - 1]  # [B]
        hits = ub >= t_q[:, None]  # [B, NU]

        # ---- exact rescan of hit units ------------------------------------
        cand_q = [[] for _ in range(B)]
        max_dev = 0.0
        act_viol = 0.0
        hit_units = np.nonzero(hits.any(axis=0))[0]
        for u in hit_units:
            vw = u_vw[u]
            if vw == 0:
                continue
            qb = u_qb[u]
            qhit = np.nonzero(hits[:, u])[0]
            qs = qhit[(qhit >= qb * 128) & (qhit < (qb + 1) * 128)]
            if qs.size == 0:
                continue
            pr = qs - qb * 128  # partition rows holding these queries
            g0 = u_col0[u]
            Xc = Xs[g0 : g0 + vw]
            Sp = tq2m[qs] @ Xc.T  # [nq, vw] exact s'
            S = Sp - xn_s[g0 : g0 + vw][None, :]
            # device-vs-exact validation
            if u_kind[u] == "D" and not partial[u]:
                d = np.max(np.abs(Sp.max(axis=1) - dev_vals[pr, u]))
                max_dev = max(max_dev, float(d))
            elif u_kind[u] == "A":
                ub_sp = (
                    U_q[qs].astype(np.float64)
                    + lnT * np.log(np.maximum(dev_vals[pr, u], 1e-300))
                    + delta
                )
                act_viol = max(
                    act_viol, float(np.max(Sp.max(axis=1) - ub_sp))
                )
            mask = S >= t_q[qs, None]
            rr, cc = np.nonzero(mask)
            for r_i, c_i in zip(rr, cc):
                cand_q[qs[r_i]].append(
                    (float(S[r_i, c_i]), int(order[g0 + c_i]))
                )

        ok = (
            max_dev <= delta - 0.05
            and act_viol <= 0.02
            and all(len(lst) >= K for lst in cand_q)
        )
        if ok:
            break
        delta = max(2.0 * delta, 2.5 * max_dev + 0.1)

    post_idx = np.empty((B, K), dtype=np.int64)
    for q in range(B):
        lst = cand_q[q]
        assert len(lst) >= K, f"query {q}: only {len(lst)} candidates"
        lst.sort(key=lambda vc: (-vc[0], vc[1]))
        post_idx[q] = [gi for _, gi in lst[:K]]

    # ---- final loss (tiny), mirroring the reference math ------------------
    T_qm = q_batch @ W  # [B, D] fp32
    X_nb = X[post_idx]  # [B, K, D]
    diff = T_qm[:, None, :] - X_nb
    l2 = np.sum(diff * diff, axis=-1, dtype=np.float32)  # [B, K]
    post_w = _softmax_f32(-l2 / np.float32(TAU))  # [B, K]

    pre_idx_b = pre_indices[q_indices]  # [B, K]
    pre_w_b = pre_weights[q_indices]  # [B, K]

    p_dense = np.zeros((B, N), np.float32)
    p_dense[rows, pre_idx_b] = pre_w_b
    q_dense = np.zeros((B, N), np.float32)
    q_dense[rows, post_idx] = post_w
    union = (p_dense > 0) | (q_dense > 0)
    p = np.where(union, np.maximum(p_dense, np.float32(EPS)), np.float32(0.0))
    p = p / p.sum(axis=1, keepdims=True)
    q = np.where(union, np.maximum(q_dense, np.float32(EPS)), np.float32(0.0))
    q = q / q.sum(axis=1, keepdims=True)
    logp = np.where(union, np.log(np.maximum(p, np.float32(1e-20))), np.float32(0.0))
    logq = np.where(union, np.log(np.maximum(q, np.float32(1e-20))), np.float32(0.0))
    kl = np.sum(np.where(union, p * (logp - logq), np.float32(0.0)), axis=1)
    loss_knn = np.float32(np.mean(kl))
    loss_reg = np.float32(0.5) * np.float32(np.sum(W * W))
    total_loss = np.float32(BETA) * loss_knn + np.float32(LAMB) * loss_reg
    return (
        np.float32(total_loss),
        np.float32(0.0),
        np.float32(loss_knn),
    )


# revision 15
# speedup vs baseline: 1.1757x; 1.1757x over previous
"""Trainium2 Bass kernel for nn_CustomLoss_11630771438153 (retrieval_knn).

Strategy v2: shard the database X row-wise across 8 NeuronCores, with the
database pre-sorted by ||x||^2 on the host. Each core computes the raw
inner-product score s' = tq2.T @ X_shard (tq2 = 2*(q@W).T, bf16) on the
TensorEngine -- a single matmul per 512-column PSUM bank, NO xnorm
accumulation pass (the sort makes per-unit xnorm nearly constant, so the
host folds it into the screening bounds analytically). The 3.2M score
elements per core are then screened by TWO engines in parallel, chunk
interleaved:

  - DVE `pool_max` (window 32): per-32-column block maxima of s',
    one instruction per 1024-col chunk covering both query blocks.
  - ScalarE `activation(Exp, bias=-U_q/T, scale=1/T, accum_out=...)`:
    per-512-column-group sumexp, giving log-sum-exp bounds
    max' <= U + T*ln(sum) <= max' + T*ln(512).

Both engines read PSUM directly (1 elem/cycle/lane each); running them
concurrently roughly halves the screening wall vs. the MAX8-only baseline,
and dropping the xnorm matmul pass halves TensorEngine time.

The host merges: every reported unit yields a sound upper bound UB on the
best true score in the unit and a sound lower bound LB witnessing one real
point; t_q = 16th largest LB guarantees >=16 points above it; all units
with UB >= t_q are rescanned exactly in fp32 (numpy), giving exact top-16
as long as the device-vs-exact deviation delta is covered (validated on the
rescanned units, with automatic escalation of delta when violated).
"""

import sys

sys.path.insert(0, "/opt/trn_rl_repo")

import ml_dtypes
import numpy as np

import concourse.tile as tile
from concourse import bacc, mybir
from concourse.bass_utils import run_bass_kernel_spmd

# Problem constants (hardcoded per the harness contract).
B = 256  # queries
D = 128  # feature dim
N = 100000  # database size
K = 16  # neighbors
TAU = 0.1
BETA = 1.0
LAMB = 1e-4
EPS = 1e-8

N_CORES = 8
N_CORE = N // N_CORES  # 12500 database rows per core
QB = B // 128  # 2 query blocks of 128

W_CHUNK = 1024  # columns per full chunk (per qblock)
N_FULL = 12  # full chunks per core (12*1024 = 12288)
REM = N_CORE - N_FULL * W_CHUNK  # 212 remainder columns
REM_PAD = 224  # padded remainder (multiple of 32)
N_PAD = N_FULL * W_CHUNK + REM_PAD  # 12512 padded per-core columns

POOLW = 32  # windowed-max width (host rescan granularity for DVE chunks)
SGRP = 1024  # scalar sumexp group width
T_SM = 0.4  # softmax screen temperature
U_PAD = 15.0  # safety pad on the per-query shift estimate
WARMUP_MM = 18  # back-to-back N=512 dummy matmuls; >=3.4us of sustained PE
# activity is required to trip the HAM clock gate (1.2 -> 2.4 GHz)

# chunk ci -> screening engine; scalar chunks early, DVE (cheaper) late so
# the kernel tail is short. 5 scalar + 7 DVE + remainder DVE balances the
# two engines' per-chunk costs (~2.56us ACT vs ~1.9us DVE).
CHUNK_ENG = ["A", "D", "A", "D", "A", "D", "A", "D", "A", "D", "D", "D", "D"]
N_CHUNKS = len(CHUNK_ENG)
assert N_CHUNKS == N_FULL + 1

# output column layout: separate per-engine output tiles (a shared tile
# would serialize the two screen engines through tile dependencies).
# DVE chunk -> 2w/POOLW block maxes; ACT chunk -> QB*(w/SGRP) sumexps.
_offd, _offa = 0, 0
CHUNK_OUT_OFF = []
for _ci in range(N_CHUNKS):
    _w = W_CHUNK if _ci < N_FULL else REM_PAD
    if CHUNK_ENG[_ci] == "D":
        CHUNK_OUT_OFF.append(_offd)
        _offd += (2 * _w) // POOLW
    else:
        CHUNK_OUT_OFF.append(_offa)
        _offa += QB * (_w // SGRP)
OUTW_D = _offd  # 7*64 + 14 = 462
OUTW_A = _offa  # 5*2 = 10

_compiled = {}
LAST_EXEC_NS = None


def _build_kernel():
    """Build + compile the SPMD Bass program (identical on all cores)."""
    nc = bacc.Bacc(
        "TRN2", target_bir_lowering=False, debug=False, num_devices=N_CORES
    )
    f32 = mybir.dt.float32
    bf16 = mybir.dt.bfloat16

    xt = nc.dram_tensor("xt", [D, N_PAD], bf16, kind="ExternalInput").ap()
    tq2_in = nc.dram_tensor("tq2", [D, B], bf16, kind="ExternalInput").ap()
    # per-partition exp bias: col qb holds -U_q/T for query (qb*128 + p)
    negb_in = nc.dram_tensor("negb", [128, QB], f32, kind="ExternalInput").ap()
    cand_d = nc.dram_tensor(
        "cand_d", [128, OUTW_D], f32, kind="ExternalOutput"
    ).ap()
    cand_a = nc.dram_tensor(
        "cand_a", [128, OUTW_A], f32, kind="ExternalOutput"
    ).ap()

    with tile.TileContext(nc) as tc:
        with (
            tc.tile_pool(name="const", bufs=1) as const_pool,
            tc.tile_pool(name="xchunk", bufs=4) as x_pool,
            tc.tile_pool(name="escr", bufs=2) as e_pool,
            tc.tile_pool(name="out", bufs=1) as out_pool,
            tc.tile_pool(name="psum", bufs=2, space="PSUM") as psum_pool,
        ):
            tq2 = const_pool.tile([D, B], bf16)
            nc.sync.dma_start(tq2[:], tq2_in[:])
            negb = const_pool.tile([128, QB], f32)
            nc.sync.dma_start(negb[:], negb_in[:])

            out_dve = out_pool.tile([128, OUTW_D], f32, name="out_dve")
            out_act = out_pool.tile([128, OUTW_A], f32, name="out_act")

            for ci in range(N_CHUNKS):
                w = W_CHUNK if ci < N_FULL else REM_PAD
                c0 = ci * W_CHUNK
                xc = x_pool.tile([D, w], bf16, tag="xc", name=f"xc{ci}")
                nc.sync.dma_start(xc[:], xt[:, c0 : c0 + w])

                # qb1 scores start at a 512-aligned (bank-aligned) offset so
                # every matmul output region is bank-exclusive
                qb_off = w if w % 512 == 0 else 512
                ps = psum_pool.tile(
                    [128, 2 * qb_off], f32, tag="ps", name=f"ps{ci}"
                )

                if ci == 0:
                    # PE warmup during the initial DMA window: >=3.4us of
                    # back-to-back matmul activity is needed to flip the HAM
                    # clock gate to 2.4 GHz (a short burst never flips it).
                    for _ in range(WARMUP_MM):
                        nc.tensor.matmul(
                            ps[:, 0:256],
                            tq2[:, 0:128],
                            tq2[:, 0:256],
                            start=True,
                            stop=True,
                        )

                for qb in range(QB):
                    lhs = tq2[:, qb * 128 : (qb + 1) * 128]
                    for h0 in range(0, w, 512):
                        hw = min(512, w - h0)
                        nc.tensor.matmul(
                            ps[:, qb * qb_off + h0 : qb * qb_off + h0 + hw],
                            lhs,
                            xc[:, h0 : h0 + hw],
                            start=True,
                            stop=True,
                        )

                off = CHUNK_OUT_OFF[ci]
                if CHUNK_ENG[ci] == "D":
                    nblk_qb = w // POOLW
                    if w == qb_off:
                        view = ps[:, 0 : 2 * w].rearrange(
                            "p (a b) -> p a b", b=POOLW
                        )
                        nc.vector.tensor_reduce(
                            out_dve[:, off : off + 2 * nblk_qb],
                            view,
                            axis=mybir.AxisListType.X,
                            op=mybir.AluOpType.max,
                        )
                    else:
                        for qb in range(QB):
                            view = ps[
                                :, qb * qb_off : qb * qb_off + w
                            ].rearrange("p (a b) -> p a b", b=POOLW)
                            nc.vector.tensor_reduce(
                                out_dve[
                                    :,
                                    off + qb * nblk_qb : off + (qb + 1) * nblk_qb,
                                ],
                                view,
                                axis=mybir.AxisListType.X,
                                op=mybir.AluOpType.max,
                            )
                else:
                    ngrp = w // SGRP
                    escr = e_pool.tile(
                        [128, 2 * w], mybir.dt.bfloat16, tag="e", name=f"e{ci}"
                    )
                    for qb in range(QB):
                        for g in range(ngrp):
                            sl = ps[
                                :,
                                qb * qb_off + g * SGRP : qb * qb_off
                                + (g + 1) * SGRP,
                            ]
                            nc.scalar.activation(
                                escr[
                                    :,
                                    qb * w + g * SGRP : qb * w + (g + 1) * SGRP,
                                ],
                                sl,
                                mybir.ActivationFunctionType.Exp,
                                bias=negb[:, qb : qb + 1],
                                scale=1.0 / T_SM,
                                accum_out=out_act[
                                    :,
                                    off + qb * ngrp + g : off + qb * ngrp + g + 1,
                                ],
                            )

            nc.sync.dma_start(cand_d[:], out_dve[:])
            nc.sync.dma_start(cand_a[:], out_act[:])

    nc.compile()
    return nc


def _get_compiled():
    if "nc" not in _compiled:
        _compiled["nc"] = _build_kernel()
    return _compiled["nc"]


def _softmax_f32(x):
    x = x.astype(np.float32)
    m = np.max(x, axis=1, keepdims=True)
    e = np.exp(x - m)
    return e / np.sum(e, axis=1, keepdims=True)


def _unit_tables():
    """Static per-core unit tables: (kind, qb, col0, width, valid_width),
    indexed by device-output column: cols [0, OUTW_D) map to the cand_d
    tile, cols [OUTW_D, OUTW_D+OUTW_A) to the cand_a tile.

    Column offsets are within the core's padded [0, N_PAD) range; validity
    clips to N_CORE.
    """
    units_d = [None] * OUTW_D
    units_a = [None] * OUTW_A
    for ci in range(N_CHUNKS):
        w = W_CHUNK if ci < N_FULL else REM_PAD
        c0 = ci * W_CHUNK
        off = CHUNK_OUT_OFF[ci]
        if CHUNK_ENG[ci] == "D":
            nblk = (2 * w) // POOLW
            for j in range(nblk):
                qb = (j * POOLW) // w
                local = (j * POOLW) % w
                col0 = c0 + local
                vw = max(0, min(POOLW, N_CORE - col0))
                units_d[off + j] = ("D", qb, col0, POOLW, vw)
        else:
            ngrp = w // SGRP
            for qb in range(QB):
                for g in range(ngrp):
                    col0 = c0 + g * SGRP
                    vw = max(0, min(SGRP, N_CORE - col0))
                    units_a[off + qb * ngrp + g] = ("A", qb, col0, SGRP, vw)
    assert all(u is not None for u in units_d + units_a)
    return units_d + units_a


_UNITS = _unit_tables()


def kernel(q_batch, q_indices, X, W, pre_indices, pre_weights):
    q_batch = np.asarray(q_batch, dtype=np.float32)
    X = np.asarray(X, dtype=np.float32)
    W = np.asarray(W, dtype=np.float32)
    q_indices = np.asarray(q_indices)
    pre_indices = np.asarray(pre_indices)
    pre_weights = np.asarray(pre_weights, dtype=np.float32)

    # ---- host prep: xnorm-sort the database, shard, quantize ---------------
    xnorm = np.sum(X * X, axis=1, dtype=np.float32)  # [N]
    order = np.argsort(xnorm, kind="stable")  # ascending
    Xs = np.ascontiguousarray(X[order])  # [N, D] sorted
    xn_s = xnorm[order]  # [N]
    tq2m = 2.0 * (q_batch @ W)  # [B, D] fp32
    tq2_dev = np.ascontiguousarray(tq2m.T.astype(ml_dtypes.bfloat16))  # [D, B]

    # per-query shift U_q: max inner-product score over a subsample + pad
    sub = Xs[:: max(1, N // 4096)]
    U_q = (tq2m @ sub.T).max(axis=1).astype(np.float32) + np.float32(U_PAD)
    negb = np.ascontiguousarray(
        (-U_q / np.float32(T_SM)).reshape(QB, 128).T.astype(np.float32)
    )  # [128, QB]

    xt_full = np.zeros((D, N_CORES * N_PAD), dtype=ml_dtypes.bfloat16)
    xs_t = Xs.T.astype(ml_dtypes.bfloat16)  # [D, N]
    for c in range(N_CORES):
        xt_full[:, c * N_PAD : c * N_PAD + N_CORE] = xs_t[
            :, c * N_CORE : (c + 1) * N_CORE
        ]

    nc = _get_compiled()
    in_maps = []
    for c in range(N_CORES):
        in_maps.append(
            {
                "xt": np.ascontiguousarray(
                    xt_full[:, c * N_PAD : (c + 1) * N_PAD]
                ),
                "tq2": tq2_dev,
                "negb": negb,
            }
        )
    res = run_bass_kernel_spmd(nc, in_maps, core_ids=list(range(N_CORES)))
    global LAST_EXEC_NS
    if res.exec_time_ns is not None:
        LAST_EXEC_NS = res.exec_time_ns

    outs = [
        np.concatenate(
            [res.results[c]["cand_d"], res.results[c]["cand_a"]], axis=1
        )
        for c in range(N_CORES)
    ]  # [128, OUTW_D + OUTW_A] each

    # ---- build per-unit bounds --------------------------------------------
    # unit arrays over all cores: shape [B, NU_total]
    nu = len(_UNITS)
    ub = np.full((B, N_CORES * nu), -np.inf, dtype=np.float64)
    lb = np.full((B, N_CORES * nu), -np.inf, dtype=np.float64)
    # metadata per global unit
    u_core = np.repeat(np.arange(N_CORES), nu)
    u_col0 = np.tile(np.array([u[2] for u in _UNITS]), N_CORES) + u_core * N_CORE
    u_vw = np.tile(np.array([u[4] for u in _UNITS]), N_CORES)
    u_kind = np.tile(np.array([u[0] for u in _UNITS]), N_CORES)
    u_qb = np.tile(np.array([u[1] for u in _UNITS]), N_CORES)

    # device values per unit, per query-in-block: dv[q_in_block, unit]
    lnT = np.float64(T_SM)
    ln_w = np.log(np.float64(SGRP))
    dev_vals = np.concatenate(
        [o.astype(np.float64) for o in outs], axis=1
    )  # [128, N_CORES*nu]

    # xnorm bounds per unit (sorted -> min/max at ends); pad cols excluded
    col0 = u_col0
    colend = u_col0 + u_vw - 1
    valid = u_vw > 0
    xmin2 = np.where(valid, xn_s[np.clip(col0, 0, N - 1)], 0.0)
    xmax2 = np.where(valid, xn_s[np.clip(colend, 0, N - 1)], 0.0)

    delta = 0.5  # device-vs-exact score deviation budget (escalated on fail)

    is_pool = u_kind == "D"
    is_act = ~is_pool
    partial = u_vw < np.where(is_pool, POOLW, SGRP)  # pad-touching units

    rows = np.arange(B)[:, None]

    for _attempt in range(5):
        for qb in range(QB):
            qsl = slice(qb * 128, (qb + 1) * 128)
            m = u_qb == qb
            mp = m & is_pool & valid
            ma = m & is_act & valid
            dv = dev_vals[:, mp]
            ub_q = np.full((128, nu * N_CORES), -np.inf)
            lb_q = np.full((128, nu * N_CORES), -np.inf)
            # pool units: dv = block max of s'
            ub_q[:, mp] = dv + delta - xmin2[mp][None, :]
            lb_q[:, mp] = dv - delta - xmax2[mp][None, :]
            # act units: dv = sumexp((s' - U_q)/T)
            dva = dev_vals[:, ma]
            with np.errstate(divide="ignore"):
                lns = np.log(dva)
            Uq = U_q[qsl].astype(np.float64)[:, None]
            ub_q[:, ma] = Uq + lnT * lns + delta - xmin2[ma][None, :]
            lb_q[:, ma] = Uq + lnT * (lns - ln_w) - delta - xmax2[ma][None, :]
            # pad-touching pool units: device max may come from a pad column
            # (score 0) -> LB is not witnessed by a real point
            lb_q[:, m & partial] = -np.inf
            ub[qsl] = ub_q
            lb[qsl] = lb_q

        # threshold: 16th largest LB per query -> >=16 real points above it
        t_q = -np.partition(-lb, K - 1, axis=1)[:, K - 1]  # [B]
        hits = ub >= t_q[:, None]  # [B, NU]

        # ---- exact rescan of hit units ------------------------------------
        cand_q = [[] for _ in range(B)]
        max_dev = 0.0
        act_viol = 0.0
        hit_units = np.nonzero(hits.any(axis=0))[0]
        for u in hit_units:
            vw = u_vw[u]
            if vw == 0:
                continue
            qb = u_qb[u]
            qhit = np.nonzero(hits[:, u])[0]
            qs = qhit[(qhit >= qb * 128) & (qhit < (qb + 1) * 128)]
            if qs.size == 0:
                continue
            pr = qs - qb * 128  # partition rows holding these queries
            g0 = u_col0[u]
            Xc = Xs[g0 : g0 + vw]
            Sp = tq2m[qs] @ Xc.T  # [nq, vw] exact s'
            S = Sp - xn_s[g0 : g0 + vw][None, :]
            # device-vs-exact validation
            if u_kind[u] == "D" and not partial[u]:
                d = np.max(np.abs(Sp.max(axis=1) - dev_vals[pr, u]))
                max_dev = max(max_dev, float(d))
            elif u_kind[u] == "A":
                ub_sp = (
                    U_q[qs].astype(np.float64)
                    + lnT * np.log(np.maximum(dev_vals[pr, u], 1e-300))
                    + delta
                )
                act_viol = max(
                    act_viol, float(np.max(Sp.max(axis=1) - ub_sp))
                )
            mask = S >= t_q[qs, None]
            rr, cc = np.nonzero(mask)
            for r_i, c_i in zip(rr, cc):
                cand_q[qs[r_i]].append(
                    (float(S[r_i, c_i]), int(order[g0 + c_i]))
                )

        ok = (
            max_dev <= delta - 0.05
            and act_viol <= 0.02
            and all(len(lst) >= K for lst in cand_q)
        )
        if ok:
            break
        delta = max(2.0 * delta, 2.5 * max_dev + 0.1)

    post_idx = np.empty((B, K), dtype=np.int64)
    for q in range(B):
        lst = cand_q[q]
        assert len(lst) >= K, f"query {q}: only {len(lst)} candidates"
        lst.sort(key=lambda vc: (-vc[0], vc[1]))
        post_idx[q] = [gi for _, gi in lst[:K]]

    # ---- final loss (tiny), mirroring the reference math ------------------
    T_qm = q_batch @ W  # [B, D] fp32
    X_nb = X[post_idx]  # [B, K, D]
    diff = T_qm[:, None, :] - X_nb
    l2 = np.sum(diff * diff, axis=-1, dtype=np.float32)  # [B, K]
    post_w = _softmax_f32(-l2 / np.float32(TAU))  # [B, K]

    pre_idx_b = pre_indices[q_indices]  # [B, K]
    pre_w_b = pre_weights[q_indices]  # [B, K]

    p_dense = np.zeros((B, N), np.float32)
    p_dense[rows, pre_idx_b] = pre_w_b
    q_dense = np.zeros((B, N), np.float32)
    q_dense[rows, post_idx] = post_w
    union = (p_dense > 0) | (q_dense > 0)
    p = np.where(union, np.maximum(p_dense, np.float32(EPS)), np.float32(0.0))
    p = p / p.sum(axis=1, keepdims=True)
    q = np.where(union, np.maximum(q_dense, np.float32(EPS)), np.float32(0.0))
    q = q / q.sum(axis=1, keepdims=True)
    logp = np.where(union, np.log(np.maximum(p, np.float32(1e-20))), np.float32(0.0))
    logq = np.where(union, np.log(np.maximum(q, np.float32(1e-20))), np.float32(0.0))
    kl = np.sum(np.where(union, p * (logp - logq), np.float32(0.0)), axis=1)
    loss_knn = np.float32(np.mean(kl))
    loss_reg = np.float32(0.5) * np.float32(np.sum(W * W))
    total_loss = np.float32(BETA) * loss_knn + np.float32(LAMB) * loss_reg
    return (
        np.float32(total_loss),
        np.float32(0.0),
        np.float32(loss_knn),
    )


# revision 18
# speedup vs baseline: 1.2722x; 1.0820x over previous
"""Trainium2 Bass kernel for nn_CustomLoss_11630771438153 (retrieval_knn).

Strategy v2: shard the database X row-wise across 8 NeuronCores, with the
database pre-sorted by ||x||^2 on the host. Each core computes the raw
inner-product score s' = tq2.T @ X_shard (tq2 = 2*(q@W).T, bf16) on the
TensorEngine -- a single matmul per 512-column PSUM bank, NO xnorm
accumulation pass (the sort makes per-unit xnorm nearly constant, so the
host folds it into the screening bounds analytically). The 3.2M score
elements per core are then screened by TWO engines in parallel, chunk
interleaved:

  - DVE `pool_max` (window 32): per-32-column block maxima of s',
    one instruction per 1024-col chunk covering both query blocks.
  - ScalarE `activation(Exp, bias=-U_q/T, scale=1/T, accum_out=...)`:
    per-512-column-group sumexp, giving log-sum-exp bounds
    max' <= U + T*ln(sum) <= max' + T*ln(512).

Both engines read PSUM directly (1 elem/cycle/lane each); running them
concurrently roughly halves the screening wall vs. the MAX8-only baseline,
and dropping the xnorm matmul pass halves TensorEngine time.

The host merges: every reported unit yields a sound upper bound UB on the
best true score in the unit and a sound lower bound LB witnessing one real
point; t_q = 16th largest LB guarantees >=16 points above it; all units
with UB >= t_q are rescanned exactly in fp32 (numpy), giving exact top-16
as long as the device-vs-exact deviation delta is covered (validated on the
rescanned units, with automatic escalation of delta when violated).
"""

import os
import sys
import time

sys.path.insert(0, "/opt/trn_rl_repo")

MERGE_DEBUG = bool(os.environ.get("MERGE_DEBUG"))

import ml_dtypes
import numpy as np

import concourse.tile as tile
from concourse import bacc, mybir
from concourse.bass_utils import run_bass_kernel_spmd

# Problem constants (hardcoded per the harness contract).
B = 256  # queries
D = 128  # feature dim
N = 100000  # database size
K = 16  # neighbors
TAU = 0.1
BETA = 1.0
LAMB = 1e-4
EPS = 1e-8

N_CORES = 8
N_CORE = N // N_CORES  # 12500 database rows per core
QB = B // 128  # 2 query blocks of 128

W_CHUNK = 1024  # columns per full chunk (per qblock)
N_FULL = 12  # full chunks per core (12*1024 = 12288)
REM = N_CORE - N_FULL * W_CHUNK  # 212 remainder columns
REM_PAD = 224  # padded remainder (multiple of 32)
N_PAD = N_FULL * W_CHUNK + REM_PAD  # 12512 padded per-core columns

POOLW = 32  # windowed-max width (host rescan granularity for DVE chunks)
SGRP = 1024  # scalar sumexp group width
T_SM = 0.4  # softmax screen temperature
U_PAD = 15.0  # safety pad on the per-query shift estimate
WARMUP_MM = 18  # back-to-back N=512 dummy matmuls; >=3.4us of sustained PE
# activity is required to trip the HAM clock gate (1.2 -> 2.4 GHz)

# chunk ci -> screening engine; scalar chunks early, DVE (cheaper) late so
# the kernel tail is short. 5 scalar + 7 DVE + remainder DVE balances the
# two engines' per-chunk costs (~2.56us ACT vs ~1.9us DVE).
CHUNK_ENG = ["D", "A", "D", "A", "D", "A", "D", "A", "D", "A", "D", "D", "A"]
N_CHUNKS = len(CHUNK_ENG)
assert N_CHUNKS == N_FULL + 1

# output column layout: separate per-engine output tiles (a shared tile
# would serialize the two screen engines through tile dependencies).
# DVE chunk -> 2w/POOLW block maxes; ACT chunk -> QB*(w/SGRP) sumexps.
_offd, _offa = 0, 0
CHUNK_OUT_OFF = []
for _ci in range(N_CHUNKS):
    _w = W_CHUNK if _ci < N_FULL else REM_PAD
    if CHUNK_ENG[_ci] == "D":
        CHUNK_OUT_OFF.append(_offd)
        _offd += (2 * _w) // POOLW
    else:
        CHUNK_OUT_OFF.append(_offa)
        _offa += QB * max(1, _w // SGRP)
OUTW_D = _offd  # 7*64 + 14 = 462
OUTW_A = _offa  # 5*2 = 10

_compiled = {}
LAST_EXEC_NS = None


def _build_kernel():
    """Build + compile the SPMD Bass program (identical on all cores)."""
    nc = bacc.Bacc(
        "TRN2", target_bir_lowering=False, debug=False, num_devices=N_CORES
    )
    f32 = mybir.dt.float32
    bf16 = mybir.dt.bfloat16

    xt = nc.dram_tensor("xt", [D, N_PAD], bf16, kind="ExternalInput").ap()
    tq2_in = nc.dram_tensor("tq2", [D, B], bf16, kind="ExternalInput").ap()
    # per-partition exp bias: col qb holds -U_q/T for query (qb*128 + p)
    negb_in = nc.dram_tensor("negb", [128, QB], f32, kind="ExternalInput").ap()
    cand_d = nc.dram_tensor(
        "cand_d", [128, OUTW_D], f32, kind="ExternalOutput"
    ).ap()
    cand_a = nc.dram_tensor(
        "cand_a", [128, OUTW_A], f32, kind="ExternalOutput"
    ).ap()

    with tile.TileContext(nc) as tc:
        with (
            tc.tile_pool(name="const", bufs=1) as const_pool,
            tc.tile_pool(name="xchunk", bufs=4) as x_pool,
            tc.tile_pool(name="escr", bufs=2) as e_pool,
            tc.tile_pool(name="out", bufs=1) as out_pool,
            tc.tile_pool(name="psum", bufs=2, space="PSUM") as psum_pool,
        ):
            warm_t = const_pool.tile([D, 256], bf16)
            nc.gpsimd.memset(warm_t[:], 0.01)
            tq2 = const_pool.tile([D, B], bf16)
            nc.sync.dma_start(tq2[:], tq2_in[:])
            negb = const_pool.tile([128, QB], f32)
            nc.sync.dma_start(negb[:], negb_in[:])

            out_dve = out_pool.tile([128, OUTW_D], f32, name="out_dve")
            out_act = out_pool.tile([128, OUTW_A], f32, name="out_act")

            for ci in range(N_CHUNKS):
                w = W_CHUNK if ci < N_FULL else REM_PAD
                c0 = ci * W_CHUNK
                xc = x_pool.tile([D, w], bf16, tag="xc", name=f"xc{ci}")
                nc.sync.dma_start(xc[:], xt[:, c0 : c0 + w])

                # qb1 scores start at a 512-aligned (bank-aligned) offset so
                # every matmul output region is bank-exclusive
                qb_off = w if w % 512 == 0 else 512
                ps = psum_pool.tile(
                    [128, 2 * qb_off], f32, tag="ps", name=f"ps{ci}"
                )

                # HAM management: the clock gate only opens after >=3.4us
                # of sustained PE activity and re-throttles on idle windows,
                # so burn a long warmup burst on chunk 0 and keep the PE
                # topped up with two filler matmuls per chunk thereafter.
                n_fill = WARMUP_MM if ci == 0 else 2
                for _ in range(n_fill):
                    nc.tensor.matmul(
                        ps[:, 0:256],
                        warm_t[:, 0:128],
                        warm_t[:, 0:256],
                        start=True,
                        stop=True,
                    )

                for qb in range(QB):
                    lhs = tq2[:, qb * 128 : (qb + 1) * 128]
                    for h0 in range(0, w, 512):
                        hw = min(512, w - h0)
                        nc.tensor.matmul(
                            ps[:, qb * qb_off + h0 : qb * qb_off + h0 + hw],
                            lhs,
                            xc[:, h0 : h0 + hw],
                            start=True,
                            stop=True,
                        )

                off = CHUNK_OUT_OFF[ci]
                if CHUNK_ENG[ci] == "D":
                    nblk_qb = w // POOLW
                    if w == qb_off:
                        view = ps[:, 0 : 2 * w].rearrange(
                            "p (a b) -> p a b", b=POOLW
                        )
                        nc.vector.tensor_reduce(
                            out_dve[:, off : off + 2 * nblk_qb],
                            view,
                            axis=mybir.AxisListType.X,
                            op=mybir.AluOpType.max,
                        )
                    else:
                        for qb in range(QB):
                            view = ps[
                                :, qb * qb_off : qb * qb_off + w
                            ].rearrange("p (a b) -> p a b", b=POOLW)
                            nc.vector.tensor_reduce(
                                out_dve[
                                    :,
                                    off + qb * nblk_qb : off + (qb + 1) * nblk_qb,
                                ],
                                view,
                                axis=mybir.AxisListType.X,
                                op=mybir.AluOpType.max,
                            )
                else:
                    ngrp = max(1, w // SGRP)
                    gw = w // ngrp
                    escr = e_pool.tile(
                        [128, 2 * w], mybir.dt.bfloat16, tag="e", name=f"e{ci}"
                    )
                    for qb in range(QB):
                        for g in range(ngrp):
                            sl = ps[
                                :,
                                qb * qb_off + g * gw : qb * qb_off
                                + (g + 1) * gw,
                            ]
                            nc.scalar.activation(
                                escr[
                                    :,
                                    qb * w + g * gw : qb * w + (g + 1) * gw,
                                ],
                                sl,
                                mybir.ActivationFunctionType.Exp,
                                bias=negb[:, qb : qb + 1],
                                scale=1.0 / T_SM,
                                accum_out=out_act[
                                    :,
                                    off + qb * ngrp + g : off + qb * ngrp + g + 1,
                                ],
                            )

            nc.sync.dma_start(cand_d[:], out_dve[:])
            nc.sync.dma_start(cand_a[:], out_act[:])

    nc.compile()
    return nc


def _get_compiled():
    if "nc" not in _compiled:
        _compiled["nc"] = _build_kernel()
    return _compiled["nc"]


def _softmax_f32(x):
    x = x.astype(np.float32)
    m = np.max(x, axis=1, keepdims=True)
    e = np.exp(x - m)
    return e / np.sum(e, axis=1, keepdims=True)


def _unit_tables():
    """Static per-core unit tables: (kind, qb, col0, width, valid_width),
    indexed by device-output column: cols [0, OUTW_D) map to the cand_d
    tile, cols [OUTW_D, OUTW_D+OUTW_A) to the cand_a tile.

    Column offsets are within the core's padded [0, N_PAD) range; validity
    clips to N_CORE.
    """
    units_d = [None] * OUTW_D
    units_a = [None] * OUTW_A
    for ci in range(N_CHUNKS):
        w = W_CHUNK if ci < N_FULL else REM_PAD
        c0 = ci * W_CHUNK
        off = CHUNK_OUT_OFF[ci]
        if CHUNK_ENG[ci] == "D":
            nblk = (2 * w) // POOLW
            for j in range(nblk):
                qb = (j * POOLW) // w
                local = (j * POOLW) % w
                col0 = c0 + local
                vw = max(0, min(POOLW, N_CORE - col0))
                units_d[off + j] = ("D", qb, col0, POOLW, vw)
        else:
            ngrp = max(1, w // SGRP)
            gw = w // ngrp
            for qb in range(QB):
                for g in range(ngrp):
                    col0 = c0 + g * gw
                    vw = max(0, min(gw, N_CORE - col0))
                    units_a[off + qb * ngrp + g] = ("A", qb, col0, gw, vw)
    assert all(u is not None for u in units_d + units_a)
    return units_d + units_a


_UNITS = _unit_tables()


def kernel(q_batch, q_indices, X, W, pre_indices, pre_weights):
    q_batch = np.asarray(q_batch, dtype=np.float32)
    X = np.asarray(X, dtype=np.float32)
    W = np.asarray(W, dtype=np.float32)
    q_indices = np.asarray(q_indices)
    pre_indices = np.asarray(pre_indices)
    pre_weights = np.asarray(pre_weights, dtype=np.float32)

    # ---- host prep: xnorm-sort the database, shard, quantize ---------------
    xnorm = np.sum(X * X, axis=1, dtype=np.float32)  # [N]
    order = np.argsort(xnorm, kind="stable")  # ascending
    Xs = np.ascontiguousarray(X[order])  # [N, D] sorted
    xn_s = xnorm[order]  # [N]
    tq2m = 2.0 * (q_batch @ W)  # [B, D] fp32
    tq2_dev = np.ascontiguousarray(tq2m.T.astype(ml_dtypes.bfloat16))  # [D, B]

    # per-query shift U_q: max inner-product score over a subsample + pad
    sub = Xs[:: max(1, N // 4096)]
    U_q = (tq2m @ sub.T).max(axis=1).astype(np.float32) + np.float32(U_PAD)
    negb = np.ascontiguousarray(
        (-U_q / np.float32(T_SM)).reshape(QB, 128).T.astype(np.float32)
    )  # [128, QB]

    xt_full = np.zeros((D, N_CORES * N_PAD), dtype=ml_dtypes.bfloat16)
    xs_t = Xs.T.astype(ml_dtypes.bfloat16)  # [D, N]
    for c in range(N_CORES):
        xt_full[:, c * N_PAD : c * N_PAD + N_CORE] = xs_t[
            :, c * N_CORE : (c + 1) * N_CORE
        ]

    nc = _get_compiled()
    in_maps = []
    for c in range(N_CORES):
        in_maps.append(
            {
                "xt": np.ascontiguousarray(
                    xt_full[:, c * N_PAD : (c + 1) * N_PAD]
                ),
                "tq2": tq2_dev,
                "negb": negb,
            }
        )
    res = run_bass_kernel_spmd(nc, in_maps, core_ids=list(range(N_CORES)))
    global LAST_EXEC_NS
    if res.exec_time_ns is not None:
        LAST_EXEC_NS = res.exec_time_ns

    outs = [
        np.concatenate(
            [res.results[c]["cand_d"], res.results[c]["cand_a"]], axis=1
        )
        for c in range(N_CORES)
    ]  # [128, OUTW_D + OUTW_A] each

    # ---- build per-unit bounds --------------------------------------------
    # unit arrays over all cores: shape [B, NU_total]
    nu = len(_UNITS)
    ub = np.full((B, N_CORES * nu), -np.inf, dtype=np.float64)
    lb = np.full((B, N_CORES * nu), -np.inf, dtype=np.float64)
    # metadata per global unit
    u_core = np.repeat(np.arange(N_CORES), nu)
    u_col0 = np.tile(np.array([u[2] for u in _UNITS]), N_CORES) + u_core * N_CORE
    u_vw = np.tile(np.array([u[4] for u in _UNITS]), N_CORES)
    u_kind = np.tile(np.array([u[0] for u in _UNITS]), N_CORES)
    u_qb = np.tile(np.array([u[1] for u in _UNITS]), N_CORES)

    # device values per unit, per query-in-block: dv[q_in_block, unit]
    lnT = np.float64(T_SM)
    u_w = np.tile(np.array([u[3] for u in _UNITS], dtype=np.float64), N_CORES)
    dev_vals = np.concatenate(
        [o.astype(np.float64) for o in outs], axis=1
    )  # [128, N_CORES*nu]

    # xnorm bounds per unit (sorted -> min/max at ends); pad cols excluded
    col0 = u_col0
    colend = u_col0 + u_vw - 1
    valid = u_vw > 0
    xmin2 = np.where(valid, xn_s[np.clip(col0, 0, N - 1)], 0.0)
    xmax2 = np.where(valid, xn_s[np.clip(colend, 0, N - 1)], 0.0)

    delta = 0.5  # device-vs-exact score deviation budget (escalated on fail)

    is_pool = u_kind == "D"
    is_act = ~is_pool
    partial = u_vw < np.where(is_pool, POOLW, SGRP)  # pad-touching units

    rows = np.arange(B)[:, None]

    for _attempt in range(5):
        for qb in range(QB):
            qsl = slice(qb * 128, (qb + 1) * 128)
            m = u_qb == qb
            mp = m & is_pool & valid
            ma = m & is_act & valid
            dv = dev_vals[:, mp]
            ub_q = np.full((128, nu * N_CORES), -np.inf)
            lb_q = np.full((128, nu * N_CORES), -np.inf)
            # pool units: dv = block max of s'
            ub_q[:, mp] = dv + delta - xmin2[mp][None, :]
            lb_q[:, mp] = dv - delta - xmax2[mp][None, :]
            # act units: dv = sumexp((s' - U_q)/T)
            dva = dev_vals[:, ma]
            with np.errstate(divide="ignore"):
                lns = np.log(dva)
            Uq = U_q[qsl].astype(np.float64)[:, None]
            ub_q[:, ma] = Uq + lnT * lns + delta - xmin2[ma][None, :]
            lb_q[:, ma] = Uq + lnT * (lns - np.log(u_w[ma][None, :])) - delta - xmax2[ma][None, :]
            # pad-touching pool units: device max may come from a pad column
            # (score 0) -> LB is not witnessed by a real point
            lb_q[:, m & partial] = -np.inf
            ub[qsl] = ub_q
            lb[qsl] = lb_q

        # threshold: 16th largest LB per query -> >=16 real points above it
        t_q = -np.partition(-lb, K - 1, axis=1)[:, K - 1]  # [B]
        hits = ub >= t_q[:, None]  # [B, NU]

        # ---- exact rescan of hit units ------------------------------------
        t_rescan = time.time()
        cand_q = [[] for _ in range(B)]
        max_dev = 0.0
        act_viol = 0.0
        n_rescan_cols = 0
        hit_units = np.nonzero(hits.any(axis=0))[0]
        for u in hit_units:
            vw = u_vw[u]
            if vw == 0:
                continue
            qb = u_qb[u]
            qhit = np.nonzero(hits[:, u])[0]
            qs = qhit[(qhit >= qb * 128) & (qhit < (qb + 1) * 128)]
            if qs.size == 0:
                continue
            pr = qs - qb * 128  # partition rows holding these queries
            g0 = u_col0[u]
            Xc = Xs[g0 : g0 + vw]
            Sp = tq2m[qs] @ Xc.T  # [nq, vw] exact s'
            S = Sp - xn_s[g0 : g0 + vw][None, :]
            # device-vs-exact validation
            if u_kind[u] == "D" and not partial[u]:
                d = np.max(np.abs(Sp.max(axis=1) - dev_vals[pr, u]))
                max_dev = max(max_dev, float(d))
            elif u_kind[u] == "A":
                ub_sp = (
                    U_q[qs].astype(np.float64)
                    + lnT * np.log(np.maximum(dev_vals[pr, u], 1e-300))
                    + delta
                )
                act_viol = max(
                    act_viol, float(np.max(Sp.max(axis=1) - ub_sp))
                )
            n_rescan_cols += int(qs.size) * int(vw)
            rr, cc = np.nonzero(S >= t_q[qs, None])
            svals = S[rr, cc]
            gidx = order[g0 + cc]
            qq = qs[rr]
            for r_i in range(rr.size):
                cand_q[qq[r_i]].append((float(svals[r_i]), int(gidx[r_i])))

        ok = (
            max_dev <= delta - 0.05
            and act_viol <= 0.02
            and all(len(lst) >= K for lst in cand_q)
        )
        if MERGE_DEBUG:
            print(
                f"merge attempt={_attempt} delta={delta:.3f} "
                f"max_dev={max_dev:.3f} act_viol={act_viol:.3f} "
                f"hit_units={hit_units.size} rescan_cols={n_rescan_cols} "
                f"ncand_min={min(len(l) for l in cand_q)} "
                f"rescan_t={time.time() - t_rescan:.2f}s ok={ok}",
                file=sys.stderr,
            )
        if ok:
            break
        delta = max(2.0 * delta, 2.5 * max_dev + 0.1)

    post_idx = np.empty((B, K), dtype=np.int64)
    for q in range(B):
        lst = cand_q[q]
        assert len(lst) >= K, f"query {q}: only {len(lst)} candidates"
        lst.sort(key=lambda vc: (-vc[0], vc[1]))
        post_idx[q] = [gi for _, gi in lst[:K]]

    # ---- final loss (tiny), mirroring the reference math ------------------
    T_qm = q_batch @ W  # [B, D] fp32
    X_nb = X[post_idx]  # [B, K, D]
    diff = T_qm[:, None, :] - X_nb
    l2 = np.sum(diff * diff, axis=-1, dtype=np.float32)  # [B, K]
    post_w = _softmax_f32(-l2 / np.float32(TAU))  # [B, K]

    pre_idx_b = pre_indices[q_indices]  # [B, K]
    pre_w_b = pre_weights[q_indices]  # [B, K]

    p_dense = np.zeros((B, N), np.float32)
    p_dense[rows, pre_idx_b] = pre_w_b
    q_dense = np.zeros((B, N), np.float32)
    q_dense[rows, post_idx] = post_w
    union = (p_dense > 0) | (q_dense > 0)
    p = np.where(union, np.maximum(p_dense, np.float32(EPS)), np.float32(0.0))
    p = p / p.sum(axis=1, keepdims=True)
    q = np.where(union, np.maximum(q_dense, np.float32(EPS)), np.float32(0.0))
    q = q / q.sum(axis=1, keepdims=True)
    logp = np.where(union, np.log(np.maximum(p, np.float32(1e-20))), np.float32(0.0))
    logq = np.where(union, np.log(np.maximum(q, np.float32(1e-20))), np.float32(0.0))
    kl = np.sum(np.where(union, p * (logp - logq), np.float32(0.0)), axis=1)
    loss_knn = np.float32(np.mean(kl))
    loss_reg = np.float32(0.5) * np.float32(np.sum(W * W))
    total_loss = np.float32(BETA) * loss_knn + np.float32(LAMB) * loss_reg
    return (
        np.float32(total_loss),
        np.float32(0.0),
        np.float32(loss_knn),
    )


# revision 20
# speedup vs baseline: 1.5119x; 1.1884x over previous
"""Trainium2 Bass kernel for nn_CustomLoss_11630771438153 (retrieval_knn).

Strategy v2: shard the database X row-wise across 8 NeuronCores, with the
database pre-sorted by ||x||^2 on the host. Each core computes the raw
inner-product score s' = tq2.T @ X_shard (tq2 = 2*(q@W).T, bf16) on the
TensorEngine -- a single matmul per 512-column PSUM bank, NO xnorm
accumulation pass (the sort makes per-unit xnorm nearly constant, so the
host folds it into the screening bounds analytically). The 3.2M score
elements per core are then screened by TWO engines in parallel, chunk
interleaved:

  - DVE `pool_max` (window 32): per-32-column block maxima of s',
    one instruction per 1024-col chunk covering both query blocks.
  - ScalarE `activation(Exp, bias=-U_q/T, scale=1/T, accum_out=...)`:
    per-512-column-group sumexp, giving log-sum-exp bounds
    max' <= U + T*ln(sum) <= max' + T*ln(512).

Both engines read PSUM directly (1 elem/cycle/lane each); running them
concurrently roughly halves the screening wall vs. the MAX8-only baseline,
and dropping the xnorm matmul pass halves TensorEngine time.

The host merges: every reported unit yields a sound upper bound UB on the
best true score in the unit and a sound lower bound LB witnessing one real
point; t_q = 16th largest LB guarantees >=16 points above it; all units
with UB >= t_q are rescanned exactly in fp32 (numpy), giving exact top-16
as long as the device-vs-exact deviation delta is covered (validated on the
rescanned units, with automatic escalation of delta when violated).
"""

import os
import sys
import time

sys.path.insert(0, "/opt/trn_rl_repo")

MERGE_DEBUG = bool(os.environ.get("MERGE_DEBUG"))

import ml_dtypes
import numpy as np

import concourse.tile as tile
from concourse import bacc, mybir
from concourse.bass_utils import run_bass_kernel_spmd

# Problem constants (hardcoded per the harness contract).
B = 256  # queries
D = 128  # feature dim
N = 100000  # database size
K = 16  # neighbors
TAU = 0.1
BETA = 1.0
LAMB = 1e-4
EPS = 1e-8

N_CORES = 8
N_CORE = N // N_CORES  # 12500 database rows per core
QB = B // 128  # 2 query blocks of 128

W_CHUNK = 1024  # columns per full chunk (per qblock)
N_FULL = 12  # full chunks per core (12*1024 = 12288)
REM = N_CORE - N_FULL * W_CHUNK  # 212 remainder columns
REM_PAD = 224  # padded remainder (multiple of 32)
N_PAD = N_FULL * W_CHUNK + REM_PAD  # 12512 padded per-core columns

POOLW = 32  # windowed-max width (host rescan granularity for DVE chunks)
SGRP = 1024  # scalar sumexp group width
T_SM = 0.4  # softmax screen temperature
U_PAD = 15.0  # safety pad on the per-query shift estimate
WARMUP_MM = 14  # back-to-back N=512 dummy matmuls; >=3.4us of sustained PE
# activity is required to trip the HAM clock gate (1.2 -> 2.4 GHz)

# Every chunk is screened by BOTH engines in parallel: one query block's
# PSUM tile goes to the DVE (windowed max), the other to ScalarE (sumexp).
# The DVE-side query block alternates per chunk so each query gets both
# kinds of screening units over the database.
N_CHUNKS = N_FULL + 1
CHUNK_W = [W_CHUNK] * N_FULL + [REM_PAD]
CHUNK_OFF_D = []
_offd = 0
for _w in CHUNK_W:
    CHUNK_OFF_D.append(_offd)
    _offd += _w // POOLW
OUTW_D = _offd  # 12*32 + 7 = 391
OUTW_A = N_CHUNKS  # one sumexp per chunk

_compiled = {}
LAST_EXEC_NS = None


def _build_kernel():
    """Build + compile the SPMD Bass program (identical on all cores)."""
    nc = bacc.Bacc(
        "TRN2", target_bir_lowering=False, debug=False, num_devices=N_CORES
    )
    f32 = mybir.dt.float32
    bf16 = mybir.dt.bfloat16

    xt = nc.dram_tensor("xt", [D, N_PAD], bf16, kind="ExternalInput").ap()
    tq2_in = nc.dram_tensor("tq2", [D, B], bf16, kind="ExternalInput").ap()
    # per-partition exp bias: col qb holds -U_q/T for query (qb*128 + p)
    negb_in = nc.dram_tensor("negb", [128, QB], f32, kind="ExternalInput").ap()
    cand_d = nc.dram_tensor(
        "cand_d", [128, OUTW_D], f32, kind="ExternalOutput"
    ).ap()
    cand_a = nc.dram_tensor(
        "cand_a", [128, OUTW_A], f32, kind="ExternalOutput"
    ).ap()

    with tile.TileContext(nc) as tc:
        with (
            tc.tile_pool(name="const", bufs=1) as const_pool,
            tc.tile_pool(name="xchunk", bufs=4) as x_pool,
            tc.tile_pool(name="escr", bufs=2) as e_pool,
            tc.tile_pool(name="out", bufs=1) as out_pool,
            tc.tile_pool(name="psum", bufs=4, space="PSUM") as psum_pool,
        ):
            warm_t = const_pool.tile([D, 256], bf16)
            nc.gpsimd.memset(warm_t[:], 0.01)
            tq2 = const_pool.tile([D, B], bf16)
            nc.sync.dma_start(tq2[:], tq2_in[:])
            negb = const_pool.tile([128, QB], f32)
            nc.sync.dma_start(negb[:], negb_in[:])

            out_dve = out_pool.tile([128, OUTW_D], f32, name="out_dve")
            out_act = out_pool.tile([128, OUTW_A], f32, name="out_act")

            for ci in range(N_CHUNKS):
                w = CHUNK_W[ci]
                c0 = ci * W_CHUNK
                xc = x_pool.tile([D, w], bf16, tag="xc", name=f"xc{ci}")
                nc.sync.dma_start(xc[:], xt[:, c0 : c0 + w])

                psq = [
                    psum_pool.tile([128, w], f32, tag="ps", name=f"ps{ci}_{qb}")
                    for qb in range(QB)
                ]

                # HAM management: the clock gate only opens after >=3.4us of
                # sustained PE activity and re-throttles on idle windows, so
                # burn a long warmup burst on chunk 0 and keep the PE topped
                # up with two filler matmuls per chunk thereafter.
                fw = min(256, w)
                for _ in range(WARMUP_MM if ci == 0 else 2):
                    nc.tensor.matmul(
                        psq[0][:, 0:fw],
                        warm_t[:, 0:128],
                        warm_t[:, 0:fw],
                        start=True,
                        stop=True,
                    )

                for qb in range(QB):
                    lhs = tq2[:, qb * 128 : (qb + 1) * 128]
                    for h0 in range(0, w, 512):
                        hw = min(512, w - h0)
                        nc.tensor.matmul(
                            psq[qb][:, h0 : h0 + hw],
                            lhs,
                            xc[:, h0 : h0 + hw],
                            start=True,
                            stop=True,
                        )

                dve_qb = ci % 2
                offd = CHUNK_OFF_D[ci]
                nblk = w // POOLW
                view = psq[dve_qb][:, 0:w].rearrange(
                    "p (a b) -> p a b", b=POOLW
                )
                nc.vector.tensor_reduce(
                    out_dve[:, offd : offd + nblk],
                    view,
                    axis=mybir.AxisListType.X,
                    op=mybir.AluOpType.max,
                )

                aqb = 1 - dve_qb
                escr = e_pool.tile(
                    [128, w], mybir.dt.bfloat16, tag="e", name=f"e{ci}"
                )
                nc.scalar.activation(
                    escr[:],
                    psq[aqb][:, 0:w],
                    mybir.ActivationFunctionType.Exp,
                    bias=negb[:, aqb : aqb + 1],
                    scale=1.0 / T_SM,
                    accum_out=out_act[:, ci : ci + 1],
                )

            nc.sync.dma_start(cand_d[:], out_dve[:])
            nc.sync.dma_start(cand_a[:], out_act[:])

    nc.compile()
    return nc


def _get_compiled():
    if "nc" not in _compiled:
        _compiled["nc"] = _build_kernel()
    return _compiled["nc"]


def _softmax_f32(x):
    x = x.astype(np.float32)
    m = np.max(x, axis=1, keepdims=True)
    e = np.exp(x - m)
    return e / np.sum(e, axis=1, keepdims=True)


def _unit_tables():
    """Static per-core unit tables: (kind, qb, col0, width, valid_width),
    indexed by device-output column: cols [0, OUTW_D) map to the cand_d
    tile, cols [OUTW_D, OUTW_D+OUTW_A) to the cand_a tile."""
    units_d = []
    units_a = []
    for ci in range(N_CHUNKS):
        w = CHUNK_W[ci]
        c0 = ci * W_CHUNK
        dve_qb = ci % 2
        for j in range(w // POOLW):
            col0 = c0 + j * POOLW
            vw = max(0, min(POOLW, N_CORE - col0))
            units_d.append(("D", dve_qb, col0, POOLW, vw))
        vw = max(0, min(w, N_CORE - c0))
        units_a.append(("A", 1 - dve_qb, c0, w, vw))
    assert len(units_d) == OUTW_D and len(units_a) == OUTW_A
    return units_d + units_a


_UNITS = _unit_tables()


def kernel(q_batch, q_indices, X, W, pre_indices, pre_weights):
    q_batch = np.asarray(q_batch, dtype=np.float32)
    X = np.asarray(X, dtype=np.float32)
    W = np.asarray(W, dtype=np.float32)
    q_indices = np.asarray(q_indices)
    pre_indices = np.asarray(pre_indices)
    pre_weights = np.asarray(pre_weights, dtype=np.float32)

    # ---- host prep: xnorm-sort the database, shard, quantize ---------------
    xnorm = np.sum(X * X, axis=1, dtype=np.float32)  # [N]
    order = np.argsort(xnorm, kind="stable")  # ascending
    Xs = np.ascontiguousarray(X[order])  # [N, D] sorted
    xn_s = xnorm[order]  # [N]
    tq2m = 2.0 * (q_batch @ W)  # [B, D] fp32
    tq2_dev = np.ascontiguousarray(tq2m.T.astype(ml_dtypes.bfloat16))  # [D, B]

    # per-query shift U_q: max inner-product score over a subsample + pad
    sub = Xs[:: max(1, N // 4096)]
    U_q = (tq2m @ sub.T).max(axis=1).astype(np.float32) + np.float32(U_PAD)
    negb = np.ascontiguousarray(
        (-U_q / np.float32(T_SM)).reshape(QB, 128).T.astype(np.float32)
    )  # [128, QB]

    xt_full = np.zeros((D, N_CORES * N_PAD), dtype=ml_dtypes.bfloat16)
    xs_t = Xs.T.astype(ml_dtypes.bfloat16)  # [D, N]
    for c in range(N_CORES):
        xt_full[:, c * N_PAD : c * N_PAD + N_CORE] = xs_t[
            :, c * N_CORE : (c + 1) * N_CORE
        ]

    nc = _get_compiled()
    in_maps = []
    for c in range(N_CORES):
        in_maps.append(
            {
                "xt": np.ascontiguousarray(
                    xt_full[:, c * N_PAD : (c + 1) * N_PAD]
                ),
                "tq2": tq2_dev,
                "negb": negb,
            }
        )
    res = run_bass_kernel_spmd(nc, in_maps, core_ids=list(range(N_CORES)))
    global LAST_EXEC_NS
    if res.exec_time_ns is not None:
        LAST_EXEC_NS = res.exec_time_ns

    outs = [
        np.concatenate(
            [res.results[c]["cand_d"], res.results[c]["cand_a"]], axis=1
        )
        for c in range(N_CORES)
    ]  # [128, OUTW_D + OUTW_A] each

    # ---- build per-unit bounds --------------------------------------------
    # unit arrays over all cores: shape [B, NU_total]
    nu = len(_UNITS)
    ub = np.full((B, N_CORES * nu), -np.inf, dtype=np.float64)
    lb = np.full((B, N_CORES * nu), -np.inf, dtype=np.float64)
    # metadata per global unit
    u_core = np.repeat(np.arange(N_CORES), nu)
    u_col0 = np.tile(np.array([u[2] for u in _UNITS]), N_CORES) + u_core * N_CORE
    u_vw = np.tile(np.array([u[4] for u in _UNITS]), N_CORES)
    u_kind = np.tile(np.array([u[0] for u in _UNITS]), N_CORES)
    u_qb = np.tile(np.array([u[1] for u in _UNITS]), N_CORES)

    # device values per unit, per query-in-block: dv[q_in_block, unit]
    lnT = np.float64(T_SM)
    u_w = np.tile(np.array([u[3] for u in _UNITS], dtype=np.float64), N_CORES)
    dev_vals = np.concatenate(
        [o.astype(np.float64) for o in outs], axis=1
    )  # [128, N_CORES*nu]

    # xnorm bounds per unit (sorted -> min/max at ends); pad cols excluded
    col0 = u_col0
    colend = u_col0 + u_vw - 1
    valid = u_vw > 0
    xmin2 = np.where(valid, xn_s[np.clip(col0, 0, N - 1)], 0.0)
    xmax2 = np.where(valid, xn_s[np.clip(colend, 0, N - 1)], 0.0)

    delta = 0.5  # device-vs-exact score deviation budget (escalated on fail)

    is_pool = u_kind == "D"
    is_act = ~is_pool
    partial = u_vw < np.array([u[3] for u in _UNITS * N_CORES])  # pad-touching

    rows = np.arange(B)[:, None]

    for _attempt in range(5):
        for qb in range(QB):
            qsl = slice(qb * 128, (qb + 1) * 128)
            m = u_qb == qb
            mp = m & is_pool & valid
            ma = m & is_act & valid
            dv = dev_vals[:, mp]
            ub_q = np.full((128, nu * N_CORES), -np.inf)
            lb_q = np.full((128, nu * N_CORES), -np.inf)
            # pool units: dv = block max of s'
            ub_q[:, mp] = dv + delta - xmin2[mp][None, :]
            lb_q[:, mp] = dv - delta - xmax2[mp][None, :]
            # act units: dv = sumexp((s' - U_q)/T)
            dva = dev_vals[:, ma]
            with np.errstate(divide="ignore"):
                lns = np.log(dva)
            Uq = U_q[qsl].astype(np.float64)[:, None]
            ub_q[:, ma] = Uq + lnT * lns + delta - xmin2[ma][None, :]
            lb_q[:, ma] = Uq + lnT * (lns - np.log(u_w[ma][None, :])) - delta - xmax2[ma][None, :]
            # pad-touching units: the device value may be influenced by a
            # pad column (score 0) -> LB is not witnessed by a real point
            lb_q[:, m & partial] = -np.inf
            ub[qsl] = ub_q
            lb[qsl] = lb_q

        # threshold: 16th largest LB per query -> >=16 real points above it
        t_q = -np.partition(-lb, K - 1, axis=1)[:, K - 1]  # [B]
        hits = ub >= t_q[:, None]  # [B, NU]

        # ---- exact rescan of hit units ------------------------------------
        t_rescan = time.time()
        cand_q = [[] for _ in range(B)]
        max_dev = 0.0
        act_viol = 0.0
        n_rescan_cols = 0
        hit_units = np.nonzero(hits.any(axis=0))[0]
        for u in hit_units:
            vw = u_vw[u]
            if vw == 0:
                continue
            qb = u_qb[u]
            qhit = np.nonzero(hits[:, u])[0]
            qs = qhit[(qhit >= qb * 128) & (qhit < (qb + 1) * 128)]
            if qs.size == 0:
                continue
            pr = qs - qb * 128  # partition rows holding these queries
            g0 = u_col0[u]
            Xc = Xs[g0 : g0 + vw]
            Sp = tq2m[qs] @ Xc.T  # [nq, vw] exact s'
            S = Sp - xn_s[g0 : g0 + vw][None, :]
            # device-vs-exact validation
            if u_kind[u] == "D" and not partial[u]:
                d = np.max(np.abs(Sp.max(axis=1) - dev_vals[pr, u]))
                max_dev = max(max_dev, float(d))
            elif u_kind[u] == "A":
                ub_sp = (
                    U_q[qs].astype(np.float64)
                    + lnT * np.log(np.maximum(dev_vals[pr, u], 1e-300))
                    + delta
                )
                act_viol = max(
                    act_viol, float(np.max(Sp.max(axis=1) - ub_sp))
                )
            n_rescan_cols += int(qs.size) * int(vw)
            rr, cc = np.nonzero(S >= t_q[qs, None])
            svals = S[rr, cc]
            gidx = order[g0 + cc]
            qq = qs[rr]
            for r_i in range(rr.size):
                cand_q[qq[r_i]].append((float(svals[r_i]), int(gidx[r_i])))

        ok = (
            max_dev <= delta - 0.05
            and act_viol <= 0.02
            and all(len(lst) >= K for lst in cand_q)
        )
        if MERGE_DEBUG:
            print(
                f"merge attempt={_attempt} delta={delta:.3f} "
                f"max_dev={max_dev:.3f} act_viol={act_viol:.3f} "
                f"hit_units={hit_units.size} rescan_cols={n_rescan_cols} "
                f"ncand_min={min(len(l) for l in cand_q)} "
                f"rescan_t={time.time() - t_rescan:.2f}s ok={ok}",
                file=sys.stderr,
            )
        if ok:
            break
        delta = max(2.0 * delta, 2.5 * max_dev + 0.1)

    post_idx = np.empty((B, K), dtype=np.int64)
    for q in range(B):
        lst = cand_q[q]
        assert len(lst) >= K, f"query {q}: only {len(lst)} candidates"
        lst.sort(key=lambda vc: (-vc[0], vc[1]))
        post_idx[q] = [gi for _, gi in lst[:K]]

    # ---- final loss (tiny), mirroring the reference math ------------------
    T_qm = q_batch @ W  # [B, D] fp32
    X_nb = X[post_idx]  # [B, K, D]
    diff = T_qm[:, None, :] - X_nb
    l2 = np.sum(diff * diff, axis=-1, dtype=np.float32)  # [B, K]
    post_w = _softmax_f32(-l2 / np.float32(TAU))  # [B, K]

    pre_idx_b = pre_indices[q_indices]  # [B, K]
    pre_w_b = pre_weights[q_indices]  # [B, K]

    p_dense = np.zeros((B, N), np.float32)
    p_dense[rows, pre_idx_b] = pre_w_b
    q_dense = np.zeros((B, N), np.float32)
    q_dense[rows, post_idx] = post_w
    union = (p_dense > 0) | (q_dense > 0)
    p = np.where(union, np.maximum(p_dense, np.float32(EPS)), np.float32(0.0))
    p = p / p.sum(axis=1, keepdims=True)
    q = np.where(union, np.maximum(q_dense, np.float32(EPS)), np.float32(0.0))
    q = q / q.sum(axis=1, keepdims=True)
    logp = np.where(union, np.log(np.maximum(p, np.float32(1e-20))), np.float32(0.0))
    logq = np.where(union, np.log(np.maximum(q, np.float32(1e-20))), np.float32(0.0))
    kl = np.sum(np.where(union, p * (logp - logq), np.float32(0.0)), axis=1)
    loss_knn = np.float32(np.mean(kl))
    loss_reg = np.float32(0.5) * np.float32(np.sum(W * W))
    total_loss = np.float32(BETA) * loss_knn + np.float32(LAMB) * loss_reg
    return (
        np.float32(total_loss),
        np.float32(0.0),
        np.float32(loss_knn),
    )
